# revision 3
# baseline (speedup 1.0000x reference)
"""FFT-based 2D long convolution on 8 Trainium2 NeuronCores — v3.

vs baseline: the inverse-hf transform (s3) pairs conj-symmetric chunks.
Chunk layout: A0 holds hf=g for g in [0,128), A1 g in [128,256), B0 holds
S[(512-g)%512] for g in [0,128) (row 0 = dup of hf=0, filter row zeroed),
B1 holds S[512-g] for g in [128,256). hf=256 is excluded and gets its own
batched path. Pair (A,B) rows are partition-aligned and their inverse-DFT
consts are conjugates, so with U = P_A + P_B, D = P_A - P_B:

    V.re += Ure.cr - Dim.ci ;  V.im += Dre.ci + Uim.cr

halving s3's matmul streams (8 of 256 per plane-half instead of 16 of 512
per plane). U/D pre-adds are bf16 SBUF ops split across DVE and Pool. The
B-chunk s2 stationaries equal the A-chunk ones (same g ranges) so
ldweights dedupe across chunks.

Nyquist-h (hf=256) per wave of 8 planes: S256T[wf, b] via const-stationary
matmuls (t256 gathered from tre col 256 — tim col 256 is exactly 0), oK via
per-partition-scalar ops, R[b,w] = p256re @ gc + p256im @ gsn (reusing s4
consts), one SBUF->SBUF DMA per plane moving R row b to partition 0, then a
contract-1 matmul py += sgn^T @ r0 fused into s4's PSUM accumulation (the
(-1)^j row reuses the pm1 const). The (hf=256, wf=256) corner cell is
dropped (~1e-4 rel err).
"""

import numpy as np
import ml_dtypes
from contextlib import ExitStack

import concourse.bass as bass
import concourse.mybir as mybir
import concourse.tile as tile
from concourse.bass_utils import run_bass_kernel_spmd

B, C, H, W = 8, 64, 256, 256
N = 512
WF = 257
WFP = 258          # even moving free size
NCORES = 8
CPC = C // NCORES
PLANES = CPC * B

F32 = mybir.dt.float32
BF16 = mybir.dt.bfloat16
NPBF16 = ml_dtypes.bfloat16


def _hfidx():
    return np.concatenate([
        np.arange(0, 128), np.arange(128, 256),
        (512 - np.arange(0, 128)) % 512, 512 - np.arange(128, 256)])


def _consts():
    h = np.arange(H, dtype=np.float64)[:, None]
    hf = np.arange(WFP, dtype=np.float64)[None, :]     # g in [0,258)
    ah = np.exp(-2j * np.pi * h * hf / N)              # [256, 258]
    w = np.arange(W, dtype=np.float64)[:, None]
    wf = np.arange(WF, dtype=np.float64)[None, :]
    aw = np.exp(-2j * np.pi * w * wf / N)              # [256, 257]
    aw = np.concatenate([aw, np.zeros((W, 1))], axis=1)  # pad to 258
    hf2 = _hfidx()[:, None].astype(np.float64)
    h2 = np.arange(H, dtype=np.float64)[None, :]
    bh = np.exp(+2j * np.pi * hf2 * h2 / N)            # [512, 256] vnyq consts
    c = np.full((WF, 1), 2.0); c[0] = 1.0; c[256] = 1.0
    wf2 = np.arange(WF, dtype=np.float64)[:, None]
    w2 = np.arange(W, dtype=np.float64)[None, :]
    gc = c * np.cos(2 * np.pi * wf2 * w2 / N)          # [257, 256]
    gs = c * np.sin(2 * np.pi * wf2 * w2 / N)
    # s3 pair consts: rows g (pair 0: g in [0,128), pair 1: [128,256))
    g3 = np.arange(256, dtype=np.float64)[:, None]
    crp = np.cos(2 * np.pi * g3 * h2 / N)              # [256, 256]
    cip = np.sin(2 * np.pi * g3 * h2 / N)
    f = NPBF16
    bhri = np.concatenate([bh.real, bh.imag], axis=1)     # [512, 512]
    bhnr = np.concatenate([-bh.imag, bh.real], axis=1)    # [512, 512]
    bhnr[256:512] *= -1.0    # B-chunk pim is stored negated (K.im folded)
    d = {
        "ahr": (f(ah.real), 2), "ahi": (f(ah.imag), 2),
        "awr": (f(aw.real), 2), "awi": (f(aw.imag), 2), "awin": (f(-aw.imag), 2),
        "bhri": (f(bhri), 4), "bhnr": (f(bhnr), 4),
        "gc": (f(gc[:256]), 2), "gsn": (f(-gs[:256]), 2),
        "crp": (f(crp), 2), "cip": (f(cip), 2), "cin": (f(-cip), 2),
    }
    cols, offs, off = [], {}, 0
    for k, (arr, kt) in d.items():
        fd = arr.shape[1]
        cols.append(arr.reshape(kt, 128, fd).transpose(1, 0, 2).reshape(128, kt * fd))
        offs[k] = (off, fd)
        off += kt * fd
    # wf=256 G row, replicated on all partitions; its first 128 cols are
    # (-1)^j and double as the NyqH apply stationary.
    pm1 = np.tile(f(gc[256])[None, :], (128, 1))
    cols.append(pm1)
    offs["pm1"] = (off, W)
    return np.concatenate(cols, axis=1), offs


def _legalize_waits(nc, max_waits=1):
    """Split extra sem waits onto same-engine NOPs (walrus allows 1/instr)."""
    k = 0
    for fn in nc.m.functions:
        for bb in fn.blocks:
            new = []
            for ins in bb.instructions:
                si = ins.sync_info
                waits = list(si.on_wait) if (si and si.on_wait) else []
                if len(waits) > max_waits:
                    for w in waits[:-max_waits]:
                        k += 1
                        new.append(mybir.InstNoOp(
                            name=f"{ins.name}-lw{k}", engine=ins.engine,
                            ins=[], outs=[],
                            sync_info=mybir.SyncInfo(on_wait=[w], on_update=[])))
                    ins.sync_info = mybir.SyncInfo(
                        on_wait=waits[-max_waits:],
                        on_update=list(si.on_update or []))
                new.append(ins)
            bb.instructions = new
    return k


def _dedupe_ldweights(nc):
    """Remove redundant InstLdweights: legalization emits one per matmul;
    when consecutive PE matmuls share the identical stationary AP the PE
    array still holds the weights, so the reload is dead. Waits/updates of
    a removed ldweights are merged into the next instruction (the paired
    matmul); run _legalize_waits afterwards to re-split them."""
    def sig(ins):
        w = ins.ins[0]
        mr = w.memref
        return (mr.name if hasattr(mr, "name") else str(mr),
                w.offset, str(w.ap), str(w.dtype),
                getattr(ins, "is_transpose", None),
                str(getattr(ins, "perf_mode", None)),
                tuple(ins.tile_position or ()), tuple(ins.tile_size or ()))
    n = 0
    for fn in nc.m.functions:
        for bb in fn.blocks:
            new_insts = []
            last = None
            pend_w, pend_u = [], []
            for ins in bb.instructions:
                if isinstance(ins, mybir.InstLdweights):
                    s = sig(ins)
                    if last is not None and s == last:
                        si = ins.sync_info
                        if si:
                            pend_w.extend(si.on_wait or [])
                            pend_u.extend(si.on_update or [])
                        n += 1
                        continue
                    last = s
                elif isinstance(ins, mybir.InstMatmult):
                    pass
                if pend_w or pend_u:
                    si = ins.sync_info
                    w = list(si.on_wait or []) if si else []
                    u = list(si.on_update or []) if si else []
                    ins.sync_info = mybir.SyncInfo(on_wait=pend_w + w,
                                                   on_update=pend_u + u)
                    pend_w, pend_u = [], []
                new_insts.append(ins)
            assert not pend_w and not pend_u
            bb.instructions = new_insts
    return n


def build_nc(n_ch=CPC, n_b=B, reps=1, debug=False):
    nc = bass.Bass(trn_type="TRN2")
    n_planes = n_ch * n_b

    xs = nc.dram_tensor("xs", [n_planes, H, W], BF16, kind="ExternalInput").ap()
    kr_d = nc.dram_tensor("kr", [128, n_ch, 4, WFP], BF16,
                          kind="ExternalInput").ap()
    ki_d = nc.dram_tensor("ki", [128, n_ch, 4, WFP], BF16,
                          kind="ExternalInput").ap()
    k2_d = nc.dram_tensor("k2", [128, n_ch, 4], F32,    # K[256,:]T chunks
                          kind="ExternalInput").ap()
    cblob_np, coffs = _consts()
    cb_d = nc.dram_tensor("cblob", list(cblob_np.shape), BF16,
                          kind="ExternalInput").ap()
    ys = nc.dram_tensor("ys", [n_planes, H, W], BF16, kind="ExternalOutput").ap()
    if debug:
        dpre = nc.dram_tensor("dpre", [128, 4, WFP], F32, kind="ExternalOutput").ap()
        dpim = nc.dram_tensor("dpim", [128, 4, WFP], F32, kind="ExternalOutput").ap()
        dud = nc.dram_tensor("dud", [128, 4, 2, WFP], F32, kind="ExternalOutput").ap()
        dvb = nc.dram_tensor("dvb", [128, 2, 2, W], F32, kind="ExternalOutput").ap()

    with tile.TileContext(nc) as tc, ExitStack() as ctx:
        const_p = ctx.enter_context(tc.tile_pool(name="const", bufs=1))
        kc_p = ctx.enter_context(tc.tile_pool(name="kc", bufs=1))
        x_p = ctx.enter_context(tc.tile_pool(name="xp", bufs=n_b + 2))
        t_p = ctx.enter_context(tc.tile_pool(name="tp", bufs=2))
        p_p = ctx.enter_context(tc.tile_pool(name="pp", bufs=2))
        ud_p = ctx.enter_context(tc.tile_pool(name="ud", bufs=3))
        v_p = ctx.enter_context(tc.tile_pool(name="vp", bufs=2))
        y_p = ctx.enter_context(tc.tile_pool(name="yp", bufs=6))
        tm_p = ctx.enter_context(tc.tile_pool(name="tm", bufs=16))
        ny_p = ctx.enter_context(tc.tile_pool(name="ny", bufs=2))
        r0_p = ctx.enter_context(tc.tile_pool(name="r0", bufs=10))
        ps1_p = ctx.enter_context(tc.tile_pool(name="ps1", bufs=2, space="PSUM"))
        psd_p = ctx.enter_context(tc.tile_pool(name="psd", bufs=4, space="PSUM"))
        ps3_p = ctx.enter_context(tc.tile_pool(name="ps3", bufs=2, space="PSUM"))

        cb = const_p.tile(list(cblob_np.shape), BF16, tag="cb")
        c1 = coffs["awr"][0]
        c2 = coffs["bhri"][0]
        nc.sync.dma_start(out=cb[:, 0:c1], in_=cb_d[:, 0:c1])
        nc.sync.dma_start(out=cb[:, c1:c2], in_=cb_d[:, c1:c2])

        class CV:
            def __init__(self, name, fd):
                self.off, self.fd = coffs[name][0], fd
            def __getitem__(self, idx):
                p, k, fs_ = idx
                lo = self.off + k * self.fd
                if fs_ == slice(None):
                    return cb[p, lo:lo + self.fd]
                return cb[p, lo + fs_.start:lo + fs_.stop]

        ahr = CV("ahr", WFP); ahi = CV("ahi", WFP)
        awr = CV("awr", WFP); awi = CV("awi", WFP); awin = CV("awin", WFP)
        bhri = CV("bhri", 2 * H); bhnr = CV("bhnr", 2 * H)
        gc = CV("gc", W); gsn = CV("gsn", W)
        crp = CV("crp", W); cip = CV("cip", W); cin = CV("cin", W)
        pm1_off = coffs["pm1"][0]
        pm1r = cb[:, pm1_off:pm1_off + W]
        sgn1 = cb[0:1, pm1_off:pm1_off + 128]          # (-1)^j row

        kre = kc_p.tile([128, n_ch, 4, WFP], BF16, tag="kre")
        kim = kc_p.tile([128, n_ch, 4, WFP], BF16, tag="kim")
        k2t = kc_p.tile([128, n_ch, 4], F32, tag="k2t")
        late_dma = [False]

        def emit_late_dmas():
            if late_dma[0]:
                return
            late_dma[0] = True
            nc.sync.dma_start(out=kre, in_=kr_d)
            nc.sync.dma_start(out=kim, in_=ki_d)
            nc.sync.dma_start(out=k2t, in_=k2_d)
            nc.sync.dma_start(out=cb[:, c2:], in_=cb_d[:, c2:])

        MM = nc.tensor.matmul
        MULT = mybir.AluOpType.mult
        ADD = mybir.AluOpType.add
        SUB = mybir.AluOpType.subtract

        CHUNK_G0 = (0, 128, 0, 128)
        CHUNK_CONJ = (False, False, True, True)

        def fwd(plane_ap, ch, b, pre_all, pim_all, t256b):
            """s1+s2+oK for one plane; writes spectrum chunks into
            pre_all/pim_all[:, b] and tre col 256 into t256b[:, :, b]."""
            xt = x_p.tile([128, 2, W], BF16, tag="xt")
            nc.sync.dma_start(out=xt, in_=plane_ap.rearrange("(k p) w -> p k w", p=128))
            emit_late_dmas()
            tre = t_p.tile([128, 2, WFP], BF16, tag="tre")
            tim = t_p.tile([128, 2, WFP], BF16, tag="tim")
            for mw in range(2):
                pr = ps1_p.tile([128, WFP], F32, tag="ps1")
                pi = ps1_p.tile([128, WFP], F32, tag="ps1")
                for kh in range(2):
                    lhsT = xt[:, kh, mw * 128:(mw + 1) * 128]
                    MM(pr, lhsT, ahr[:, kh, :], start=(kh == 0), stop=(kh == 1))
                    MM(pi, lhsT, ahi[:, kh, :], start=(kh == 0), stop=(kh == 1))
                nc.scalar.copy(out=tre[:, mw, :], in_=pr)
                nc.scalar.copy(out=tim[:, mw, :], in_=pi)
            nc.scalar.copy(out=t256b[:, :, b:b + 1], in_=tre[:, :, 256:257])
            s_all = tm_p.tile([128, 4, 2, WFP], BF16, tag="sall", bufs=2)
            for mhf in range(4):
                g0 = CHUNK_G0[mhf]
                cj = CHUNK_CONJ[mhf]
                sr = psd_p.tile([128, WFP], F32, tag="psd")
                si = psd_p.tile([128, WFP], F32, tag="psd")
                for kw in range(2):
                    lre = tre[:, kw, g0:g0 + 128]
                    lim = tim[:, kw, g0:g0 + 128]
                    MM(sr, lre, awr[:, kw, :], start=(kw == 0), stop=False)
                    MM(si, lre, (awin if cj else awi)[:, kw, :],
                       start=(kw == 0), stop=False)
                    MM(sr, lim, (awi if cj else awin)[:, kw, :],
                       start=False, stop=(kw == 1))
                    MM(si, lim, awr[:, kw, :], start=False, stop=(kw == 1))
                nc.scalar.copy(out=s_all[:, mhf, 0, :], in_=sr)
                nc.scalar.copy(out=s_all[:, mhf, 1, :], in_=si)
            # oK: fused muls over all 4 chunks, then per-pair-signed addsub
            kpr = kre[:, ch, :, :]
            kpi = kim[:, ch, :, :]
            t1 = tm_p.tile([128, 4, WFP], BF16, tag="tm4", bufs=8)
            t2 = tm_p.tile([128, 4, WFP], BF16, tag="tm4", bufs=8)
            t3 = tm_p.tile([128, 4, WFP], BF16, tag="tm4", bufs=8)
            t4 = tm_p.tile([128, 4, WFP], BF16, tag="tm4", bufs=8)
            nc.vector.tensor_mul(t1, s_all[:, :, 0, :], kpr)
            nc.vector.tensor_mul(t2, s_all[:, :, 1, :], kpi)
            nc.vector.tensor_mul(t3, s_all[:, :, 0, :], kpi)
            nc.vector.tensor_mul(t4, s_all[:, :, 1, :], kpr)
            po = pre_all[:, b]
            qo = pim_all[:, b]
            # K.im negated for B chunks on host -> uniform signs; B-chunk
            # pim is stored as -Im(P) (handled in UD im-swap + bhnr rows)
            nc.vector.tensor_sub(po, t1, t2)
            nc.vector.tensor_add(qo, t3, t4)
            return xt

        rep_ctx = tc.For_i(0, reps, 1) if reps > 1 else None
        if rep_ctx is not None:
            rep_ctx.__enter__()
        for ch in range(n_ch):
            pre_all = p_p.tile([128, n_b, 4, WFP], BF16, tag="pre")
            pim_all = p_p.tile([128, n_b, 4, WFP], BF16, tag="pim")
            t256b = ny_p.tile([128, 2, n_b], BF16, tag="t256")
            xts = []
            for b in range(n_b):
                xts.append(fwd(xs[ch * n_b + b], ch, b, pre_all, pim_all, t256b))

            # ---- NyqH (hf=256): batched transposed spectrum + R rows ----
            s256p = psd_p.tile([128, 2, 2, n_b], F32, tag="psd")
            for c in range(2):
                cs = slice(c * 128, (c + 1) * 128)
                for kw in range(2):
                    MM(s256p[:, c, 0, :], awr[:, kw, cs], t256b[:, kw, :],
                       start=(kw == 0), stop=(kw == 1))
                for kw in range(2):
                    MM(s256p[:, c, 1, :], awi[:, kw, cs], t256b[:, kw, :],
                       start=(kw == 0), stop=(kw == 1))
            s256 = ny_p.tile([128, 2, 2, n_b], BF16, tag="s256")
            nc.scalar.copy(out=s256, in_=s256p)
            p256 = ny_p.tile([128, 2, 2, n_b], BF16, tag="p256")
            for c in range(2):
                m2 = ny_p.tile([128, 2, n_b], BF16, tag="m2")
                krs = k2t[:, ch, 2 * c:2 * c + 1]       # K256T re, chunk c
                kis = k2t[:, ch, 2 * c + 1:2 * c + 2]   # K256T im, chunk c
                nc.vector.tensor_scalar_mul(m2[:, 0, :], s256[:, c, 1, :], kis)
                nc.vector.scalar_tensor_tensor(
                    p256[:, c, 0, :], s256[:, c, 0, :], krs, m2[:, 0, :],
                    MULT, SUB)
                nc.vector.tensor_scalar_mul(m2[:, 1, :], s256[:, c, 1, :], krs)
                nc.vector.scalar_tensor_tensor(
                    p256[:, c, 1, :], s256[:, c, 0, :], kis, m2[:, 1, :],
                    MULT, ADD)
            rp = psd_p.tile([n_b, W], F32, tag="psd")
            for c in range(2):
                MM(rp, p256[:, c, 0, :], gc[:, c, :],
                   start=(c == 0), stop=False)
                MM(rp, p256[:, c, 1, :], gsn[:, c, :],
                   start=False, stop=(c == 1))
            rsb = ny_p.tile([n_b, W], BF16, tag="rsb")
            nc.scalar.copy(out=rsb, in_=rp)
            r0s = []
            for b in range(n_b):
                r0 = r0_p.tile([1, W], BF16, tag="r0")
                nc.sync.dma_start(out=r0, in_=rsb[b:b + 1, :])
                r0s.append(r0)

            # ---- vnyq (wf=256): batched for the wave ----
            pvnT = psd_p.tile([128, 2, n_b], F32, tag="psd")
            for mh in range(2):
                for khf in range(4):
                    MM(pvnT[:, mh, :], bhri[:, khf, slice(mh * 128, (mh + 1) * 128)],
                       pre_all[:, :, khf, 256], start=(khf == 0), stop=False)
                    MM(pvnT[:, mh, :], bhnr[:, khf, slice(mh * 128, (mh + 1) * 128)],
                       pim_all[:, :, khf, 256], start=False, stop=(khf == 3))
            vnyqT = v_p.tile([128, 2, n_b], BF16, tag="vnyqT")
            nc.scalar.copy(out=vnyqT, in_=pvnT)

            # ---- U/D pre-adds + inverse transforms + output per plane ----
            for b in range(n_b):
                pl = ch * n_b + b
                ure = ud_p.tile([128, 2, WFP], BF16, tag="ure")
                dre = ud_p.tile([128, 2, WFP], BF16, tag="dre")
                uim = ud_p.tile([128, 2, WFP], BF16, tag="uim")
                dim = ud_p.tile([128, 2, WFP], BF16, tag="dim")
                pa_re = pre_all[:, b, 0:2, :]
                pb_re = pre_all[:, b, 2:4, :]
                pa_im = pim_all[:, b, 0:2, :]
                pb_im = pim_all[:, b, 2:4, :]
                nc.vector.tensor_add(ure, pa_re, pb_re)
                nc.vector.tensor_sub(dre, pa_re, pb_re)
                # B-chunk pim holds -Im(P): swap add/sub for the im parts
                nc.gpsimd.tensor_sub(uim, pa_im, pb_im)
                nc.gpsimd.tensor_add(dim, pa_im, pb_im)
                if debug and ch == 0 and b == 0:
                    dbt = y_p.tile([128, 4, WFP], F32, tag="dbt")
                    nc.vector.tensor_copy(dbt[:, 0:2], pre_all[:, b, 0:2, :])
                    nc.vector.tensor_copy(dbt[:, 2:4], pre_all[:, b, 2:4, :])
                    nc.sync.dma_start(out=dpre, in_=dbt)
                    dbt2 = y_p.tile([128, 4, WFP], F32, tag="dbt2")
                    nc.vector.tensor_copy(dbt2[:, 0:2], pim_all[:, b, 0:2, :])
                    nc.vector.tensor_copy(dbt2[:, 2:4], pim_all[:, b, 2:4, :])
                    nc.sync.dma_start(out=dpim, in_=dbt2)
                    dbu = y_p.tile([128, 4, 2, WFP], F32, tag="dbu")
                    nc.vector.tensor_copy(dbu[:, 0], ure)
                    nc.vector.tensor_copy(dbu[:, 1], dre)
                    nc.vector.tensor_copy(dbu[:, 2], uim)
                    nc.vector.tensor_copy(dbu[:, 3], dim)
                    nc.sync.dma_start(out=dud, in_=dbu)

                v_both = v_p.tile([128, 2, 2, W], BF16, tag="vb")
                for mwf in range(2):
                    vp = ps3_p.tile([128, 2, W], F32, tag="ps3")
                    ms = slice(mwf * 128, (mwf + 1) * 128)
                    # each psum region's accumulation run must be contiguous
                    for pair in range(2):
                        MM(vp[:, 0, :], ure[:, pair, ms], crp[:, pair, :],
                           start=(pair == 0), stop=False)
                        MM(vp[:, 0, :], dim[:, pair, ms], cin[:, pair, :],
                           start=False, stop=(pair == 1))
                    for pair in range(2):
                        MM(vp[:, 1, :], dre[:, pair, ms], cip[:, pair, :],
                           start=(pair == 0), stop=False)
                        MM(vp[:, 1, :], uim[:, pair, ms], crp[:, pair, :],
                           start=False, stop=(pair == 1))
                    nc.scalar.copy(out=v_both[:, mwf], in_=vp)
                if debug and ch == 0 and b == 0:
                    dbv = y_p.tile([128, 2, 2, W], F32, tag="dbv")
                    nc.vector.tensor_copy(dbv, v_both)
                    nc.sync.dma_start(out=dvb, in_=dbv)

                ysb = y_p.tile([128, 2, W], BF16, tag="ysb")
                xt = xts[b]
                for mh in range(2):
                    py = psd_p.tile([128, W], F32, tag="psd")
                    mhs = slice(mh * 128, (mh + 1) * 128)
                    MM(py, v_both[:, 0, 0, mhs], gc[:, 0, :],
                       start=True, stop=False)
                    MM(py, v_both[:, 0, 1, mhs], gsn[:, 0, :],
                       start=False, stop=False)
                    MM(py, v_both[:, 1, 0, mhs], gc[:, 1, :],
                       start=False, stop=False)
                    MM(py, v_both[:, 1, 1, mhs], gsn[:, 1, :],
                       start=False, stop=False)
                    MM(py, sgn1, r0s[b], start=False, stop=True)
                    tny = tm_p.tile([128, W], F32, tag="tm")
                    nc.vector.scalar_tensor_tensor(
                        tny, pm1r, vnyqT[:, mh, b:b + 1], xt[:, mh, :],
                        MULT, ADD)
                    nc.vector.tensor_add(ysb[:, mh, :], tny, py)
                nc.sync.dma_start(out=ys[pl].rearrange("(k p) w -> p k w", p=128),
                                  in_=ysb)
        if rep_ctx is not None:
            rep_ctx.__exit__(None, None, None)
    _dedupe_ldweights(nc)
    _legalize_waits(nc)
    return nc


def filter_spectra(filt_slice: np.ndarray):
    """Host rfft2 -> chunk layout [128, n_ch, 4, WFP] (re, im) with B0 row-0
    zeroed, plus K[256,:] transposed chunks [128, n_ch, 4] = (c0re, c0im,
    c1re, c1im)."""
    n_ch = filt_slice.shape[0]
    kf = np.fft.rfft2(filt_slice.astype(np.float64), s=(N, N))  # [n_ch,512,257]
    k256 = kf[:, 256, :]                                       # [n_ch, 257]
    kc = kf[:, _hfidx(), :]
    kc[:, 256] = 0.0          # duplicate hf=0 row (B0 chunk row 0)
    kk = kc.reshape(n_ch, 4, 128, WF).transpose(2, 0, 1, 3)    # [128,n_ch,4,257]
    out = np.zeros((2, 128, n_ch, 4, WFP), np.float32)
    out[0, :, :, :, :WF] = kk.real
    out[1, :, :, :, :WF] = kk.imag
    out[1, :, :, 2:4, :] *= -1.0    # fold B-chunk conj sign into K.im
    k2 = np.zeros((128, n_ch, 4), np.float32)
    k2[:, :, 0] = k256.real[:, 0:128].T
    k2[:, :, 1] = k256.imag[:, 0:128].T
    k2[:, :, 2] = k256.real[:, 128:256].T
    k2[:, :, 3] = k256.imag[:, 128:256].T
    return (out[0].astype(NPBF16), out[1].astype(NPBF16), k2)


def kernel(x: np.ndarray, filt: np.ndarray) -> np.ndarray:
    x = np.ascontiguousarray(x, dtype=np.float32)
    xb = x.astype(NPBF16)
    filt = np.ascontiguousarray(filt, dtype=np.float32)
    cblob = _consts()[0]
    nc = build_nc()
    in_maps = []
    for i in range(NCORES):
        sl = slice(i * CPC, (i + 1) * CPC)
        xsh = np.ascontiguousarray(
            xb[:, sl].transpose(1, 0, 2, 3).reshape(PLANES, H, W))
        kr, ki, k2 = filter_spectra(filt[sl])
        in_maps.append({"xs": xsh, "kr": kr, "ki": ki, "k2": k2,
                        "cblob": cblob})
    res = run_bass_kernel_spmd(nc, in_maps, core_ids=list(range(NCORES)))
    out = np.empty_like(x)
    for i in range(NCORES):
        sl = slice(i * CPC, (i + 1) * CPC)
        out[:, sl] = res.results[i]["ys"].astype(np.float32).reshape(CPC, B, H, W).transpose(1, 0, 2, 3)
    return out


# revision 4
# speedup vs baseline: 1.0144x; 1.0144x over previous
"""FFT-based 2D long convolution on 8 Trainium2 NeuronCores — v3.

vs baseline: the inverse-hf transform (s3) pairs conj-symmetric chunks.
Chunk layout: A0 holds hf=g for g in [0,128), A1 g in [128,256), B0 holds
S[(512-g)%512] for g in [0,128) (row 0 = dup of hf=0, filter row zeroed),
B1 holds S[512-g] for g in [128,256). hf=256 is excluded and gets its own
batched path. Pair (A,B) rows are partition-aligned and their inverse-DFT
consts are conjugates, so with U = P_A + P_B, D = P_A - P_B:

    V.re += Ure.cr - Dim.ci ;  V.im += Dre.ci + Uim.cr

halving s3's matmul streams (8 of 256 per plane-half instead of 16 of 512
per plane). U/D pre-adds are bf16 SBUF ops split across DVE and Pool. The
B-chunk s2 stationaries equal the A-chunk ones (same g ranges) so
ldweights dedupe across chunks.

Nyquist-h (hf=256) per wave of 8 planes: S256T[wf, b] via const-stationary
matmuls (t256 gathered from tre col 256 — tim col 256 is exactly 0), oK via
per-partition-scalar ops, R[b,w] = p256re @ gc + p256im @ gsn (reusing s4
consts), one SBUF->SBUF DMA per plane moving R row b to partition 0, then a
contract-1 matmul py += sgn^T @ r0 fused into s4's PSUM accumulation (the
(-1)^j row reuses the pm1 const). The (hf=256, wf=256) corner cell is
dropped (~1e-4 rel err).
"""

import numpy as np
import ml_dtypes
from contextlib import ExitStack

import concourse.bass as bass
import concourse.mybir as mybir
import concourse.tile as tile
from concourse.bass_utils import run_bass_kernel_spmd

B, C, H, W = 8, 64, 256, 256
N = 512
WF = 257
WFP = 258          # even moving free size
NCORES = 8
CPC = C // NCORES
PLANES = CPC * B

F32 = mybir.dt.float32
BF16 = mybir.dt.bfloat16
NPBF16 = ml_dtypes.bfloat16


def _hfidx():
    return np.concatenate([
        np.arange(0, 128), np.arange(128, 256),
        (512 - np.arange(0, 128)) % 512, 512 - np.arange(128, 256)])


def _consts():
    h = np.arange(H, dtype=np.float64)[:, None]
    hf = np.arange(WFP, dtype=np.float64)[None, :]     # g in [0,258)
    ah = np.exp(-2j * np.pi * h * hf / N)              # [256, 258]
    w = np.arange(W, dtype=np.float64)[:, None]
    wf = np.arange(WF, dtype=np.float64)[None, :]
    aw = np.exp(-2j * np.pi * w * wf / N)              # [256, 257]
    aw = np.concatenate([aw, np.zeros((W, 1))], axis=1)  # pad to 258
    hf2 = _hfidx()[:, None].astype(np.float64)
    h2 = np.arange(H, dtype=np.float64)[None, :]
    bh = np.exp(+2j * np.pi * hf2 * h2 / N)            # [512, 256] vnyq consts
    c = np.full((WF, 1), 2.0); c[0] = 1.0; c[256] = 1.0
    wf2 = np.arange(WF, dtype=np.float64)[:, None]
    w2 = np.arange(W, dtype=np.float64)[None, :]
    gc = c * np.cos(2 * np.pi * wf2 * w2 / N)          # [257, 256]
    gs = c * np.sin(2 * np.pi * wf2 * w2 / N)
    # s3 pair consts: rows g (pair 0: g in [0,128), pair 1: [128,256))
    g3 = np.arange(256, dtype=np.float64)[:, None]
    crp = np.cos(2 * np.pi * g3 * h2 / N)              # [256, 256]
    cip = np.sin(2 * np.pi * g3 * h2 / N)
    f = NPBF16
    bhri = np.concatenate([bh.real, bh.imag], axis=1)     # [512, 512]
    bhnr = np.concatenate([-bh.imag, bh.real], axis=1)    # [512, 512]
    bhnr[256:512] *= -1.0    # B-chunk pim is stored negated (K.im folded)
    d = {
        "ahr": (f(ah.real), 2), "ahi": (f(ah.imag), 2),
        "awr": (f(aw.real), 2), "awi": (f(aw.imag), 2), "awin": (f(-aw.imag), 2),
        "bhri": (f(bhri), 4), "bhnr": (f(bhnr), 4),
        "gc": (f(gc[:256]), 2), "gsn": (f(-gs[:256]), 2),
        "crp": (f(crp), 2), "cip": (f(cip), 2), "cin": (f(-cip), 2),
    }
    cols, offs, off = [], {}, 0
    for k, (arr, kt) in d.items():
        fd = arr.shape[1]
        cols.append(arr.reshape(kt, 128, fd).transpose(1, 0, 2).reshape(128, kt * fd))
        offs[k] = (off, fd)
        off += kt * fd
    # wf=256 G row, replicated on all partitions; its first 128 cols are
    # (-1)^j and double as the NyqH apply stationary.
    pm1 = np.tile(f(gc[256])[None, :], (128, 1))
    cols.append(pm1)
    offs["pm1"] = (off, W)
    return np.concatenate(cols, axis=1), offs


def _legalize_waits(nc, max_waits=1):
    """Split extra sem waits onto same-engine NOPs (walrus allows 1/instr)."""
    k = 0
    for fn in nc.m.functions:
        for bb in fn.blocks:
            new = []
            for ins in bb.instructions:
                si = ins.sync_info
                waits = list(si.on_wait) if (si and si.on_wait) else []
                if len(waits) > max_waits:
                    for w in waits[:-max_waits]:
                        k += 1
                        new.append(mybir.InstNoOp(
                            name=f"{ins.name}-lw{k}", engine=ins.engine,
                            ins=[], outs=[],
                            sync_info=mybir.SyncInfo(on_wait=[w], on_update=[])))
                    ins.sync_info = mybir.SyncInfo(
                        on_wait=waits[-max_waits:],
                        on_update=list(si.on_update or []))
                new.append(ins)
            bb.instructions = new
    return k


def _dedupe_ldweights(nc):
    """Remove redundant InstLdweights: legalization emits one per matmul;
    when consecutive PE matmuls share the identical stationary AP the PE
    array still holds the weights, so the reload is dead. Waits/updates of
    a removed ldweights are merged into the next instruction (the paired
    matmul); run _legalize_waits afterwards to re-split them."""
    def sig(ins):
        w = ins.ins[0]
        mr = w.memref
        return (mr.name if hasattr(mr, "name") else str(mr),
                w.offset, str(w.ap), str(w.dtype),
                getattr(ins, "is_transpose", None),
                str(getattr(ins, "perf_mode", None)),
                tuple(ins.tile_position or ()), tuple(ins.tile_size or ()))
    n = 0
    for fn in nc.m.functions:
        for bb in fn.blocks:
            new_insts = []
            last = None
            pend_w, pend_u = [], []
            for ins in bb.instructions:
                if isinstance(ins, mybir.InstLdweights):
                    s = sig(ins)
                    if last is not None and s == last:
                        si = ins.sync_info
                        if si:
                            pend_w.extend(si.on_wait or [])
                            pend_u.extend(si.on_update or [])
                        n += 1
                        continue
                    last = s
                elif isinstance(ins, mybir.InstMatmult):
                    pass
                if pend_w or pend_u:
                    si = ins.sync_info
                    w = list(si.on_wait or []) if si else []
                    u = list(si.on_update or []) if si else []
                    ins.sync_info = mybir.SyncInfo(on_wait=pend_w + w,
                                                   on_update=pend_u + u)
                    pend_w, pend_u = [], []
                new_insts.append(ins)
            assert not pend_w and not pend_u
            bb.instructions = new_insts
    return n


def build_nc(n_ch=CPC, n_b=B, reps=1, debug=False):
    nc = bass.Bass(trn_type="TRN2")
    n_planes = n_ch * n_b

    xs = nc.dram_tensor("xs", [n_planes, H, W], BF16, kind="ExternalInput").ap()
    kr_d = nc.dram_tensor("kr", [128, n_ch, 4, WFP], BF16,
                          kind="ExternalInput").ap()
    ki_d = nc.dram_tensor("ki", [128, n_ch, 4, WFP], BF16,
                          kind="ExternalInput").ap()
    k2_d = nc.dram_tensor("k2", [128, n_ch, 4], F32,    # K[256,:]T chunks
                          kind="ExternalInput").ap()
    cblob_np, coffs = _consts()
    cb_d = nc.dram_tensor("cblob", list(cblob_np.shape), BF16,
                          kind="ExternalInput").ap()
    ys = nc.dram_tensor("ys", [n_planes, H, W], BF16, kind="ExternalOutput").ap()
    if debug:
        dpre = nc.dram_tensor("dpre", [128, 4, WFP], F32, kind="ExternalOutput").ap()
        dpim = nc.dram_tensor("dpim", [128, 4, WFP], F32, kind="ExternalOutput").ap()
        dud = nc.dram_tensor("dud", [128, 4, 2, WFP], F32, kind="ExternalOutput").ap()
        dvb = nc.dram_tensor("dvb", [128, 2, 2, W], F32, kind="ExternalOutput").ap()

    with tile.TileContext(nc) as tc, ExitStack() as ctx:
        const_p = ctx.enter_context(tc.tile_pool(name="const", bufs=1))
        kc_p = ctx.enter_context(tc.tile_pool(name="kc", bufs=1))
        x_p = ctx.enter_context(tc.tile_pool(name="xp", bufs=n_b + 2))
        t_p = ctx.enter_context(tc.tile_pool(name="tp", bufs=2))
        p_p = ctx.enter_context(tc.tile_pool(name="pp", bufs=2))
        ud_p = ctx.enter_context(tc.tile_pool(name="ud", bufs=2))
        v_p = ctx.enter_context(tc.tile_pool(name="vp", bufs=2))
        y_p = ctx.enter_context(tc.tile_pool(name="yp", bufs=4))
        tm_p = ctx.enter_context(tc.tile_pool(name="tm", bufs=16))
        ny_p = ctx.enter_context(tc.tile_pool(name="ny", bufs=2))
        r0_p = ctx.enter_context(tc.tile_pool(name="r0", bufs=10))
        ps1_p = ctx.enter_context(tc.tile_pool(name="ps1", bufs=2, space="PSUM"))
        psd_p = ctx.enter_context(tc.tile_pool(name="psd", bufs=4, space="PSUM"))
        ps3_p = ctx.enter_context(tc.tile_pool(name="ps3", bufs=2, space="PSUM"))

        cb = const_p.tile(list(cblob_np.shape), BF16, tag="cb")
        c1 = coffs["awr"][0]
        c2 = coffs["bhri"][0]
        nc.sync.dma_start(out=cb[:, 0:c1], in_=cb_d[:, 0:c1])
        nc.sync.dma_start(out=cb[:, c1:c2], in_=cb_d[:, c1:c2])

        class CV:
            def __init__(self, name, fd):
                self.off, self.fd = coffs[name][0], fd
            def __getitem__(self, idx):
                p, k, fs_ = idx
                lo = self.off + k * self.fd
                if fs_ == slice(None):
                    return cb[p, lo:lo + self.fd]
                return cb[p, lo + fs_.start:lo + fs_.stop]

        ahr = CV("ahr", WFP); ahi = CV("ahi", WFP)
        awr = CV("awr", WFP); awi = CV("awi", WFP); awin = CV("awin", WFP)
        bhri = CV("bhri", 2 * H); bhnr = CV("bhnr", 2 * H)
        gc = CV("gc", W); gsn = CV("gsn", W)
        crp = CV("crp", W); cip = CV("cip", W); cin = CV("cin", W)
        pm1_off = coffs["pm1"][0]
        pm1r = cb[:, pm1_off:pm1_off + W]
        sgn1 = cb[0:1, pm1_off:pm1_off + 128]          # (-1)^j row

        kre = kc_p.tile([128, n_ch, 4, WFP], BF16, tag="kre")
        kim = kc_p.tile([128, n_ch, 4, WFP], BF16, tag="kim")
        k2t = kc_p.tile([128, n_ch, 4], F32, tag="k2t")
        late_dma = [False]

        def emit_late_dmas():
            if late_dma[0]:
                return
            late_dma[0] = True
            nc.sync.dma_start(out=kre, in_=kr_d)
            nc.sync.dma_start(out=kim, in_=ki_d)
            nc.sync.dma_start(out=k2t, in_=k2_d)
            nc.sync.dma_start(out=cb[:, c2:], in_=cb_d[:, c2:])

        MM = nc.tensor.matmul
        MULT = mybir.AluOpType.mult
        ADD = mybir.AluOpType.add
        SUB = mybir.AluOpType.subtract

        CHUNK_G0 = (0, 128, 0, 128)
        CHUNK_CONJ = (False, False, True, True)

        def fwd(plane_ap, ch, b, pre_all, pim_all, t256b):
            """s1+s2+oK for one plane; writes spectrum chunks into
            pre_all/pim_all[:, b] and tre col 256 into t256b[:, :, b]."""
            xt = x_p.tile([128, 2, W], BF16, tag="xt")
            nc.sync.dma_start(out=xt, in_=plane_ap.rearrange("(k p) w -> p k w", p=128))
            emit_late_dmas()
            tre = t_p.tile([128, 2, WFP], BF16, tag="tre")
            tim = t_p.tile([128, 2, WFP], BF16, tag="tim")
            for mw in range(2):
                pr = ps1_p.tile([128, WFP], F32, tag="ps1")
                pi = ps1_p.tile([128, WFP], F32, tag="ps1")
                for kh in range(2):
                    lhsT = xt[:, kh, mw * 128:(mw + 1) * 128]
                    MM(pr, lhsT, ahr[:, kh, :], start=(kh == 0), stop=(kh == 1))
                    MM(pi, lhsT, ahi[:, kh, :], start=(kh == 0), stop=(kh == 1))
                nc.scalar.copy(out=tre[:, mw, :], in_=pr)
                nc.scalar.copy(out=tim[:, mw, :], in_=pi)
            nc.scalar.copy(out=t256b[:, :, b:b + 1], in_=tre[:, :, 256:257])
            s_all = tm_p.tile([128, 4, 2, WFP], BF16, tag="sall", bufs=2)
            for mhf in range(4):
                g0 = CHUNK_G0[mhf]
                cj = CHUNK_CONJ[mhf]
                sr = psd_p.tile([128, WFP], F32, tag="psd")
                si = psd_p.tile([128, WFP], F32, tag="psd")
                for kw in range(2):
                    lre = tre[:, kw, g0:g0 + 128]
                    lim = tim[:, kw, g0:g0 + 128]
                    MM(sr, lre, awr[:, kw, :], start=(kw == 0), stop=False)
                    MM(si, lre, (awin if cj else awi)[:, kw, :],
                       start=(kw == 0), stop=False)
                    MM(sr, lim, (awi if cj else awin)[:, kw, :],
                       start=False, stop=(kw == 1))
                    MM(si, lim, awr[:, kw, :], start=False, stop=(kw == 1))
                nc.scalar.copy(out=s_all[:, mhf, 0, :], in_=sr)
                nc.scalar.copy(out=s_all[:, mhf, 1, :], in_=si)
            # oK: fused muls over all 4 chunks, then per-pair-signed addsub
            kpr = kre[:, ch, :, :]
            kpi = kim[:, ch, :, :]
            t1 = tm_p.tile([128, 4, WFP], BF16, tag="tm4", bufs=8)
            t2 = tm_p.tile([128, 4, WFP], BF16, tag="tm4", bufs=8)
            t3 = tm_p.tile([128, 4, WFP], BF16, tag="tm4", bufs=8)
            t4 = tm_p.tile([128, 4, WFP], BF16, tag="tm4", bufs=8)
            nc.vector.tensor_mul(t1, s_all[:, :, 0, :], kpr)
            nc.vector.tensor_mul(t2, s_all[:, :, 1, :], kpi)
            nc.vector.tensor_mul(t3, s_all[:, :, 0, :], kpi)
            nc.vector.tensor_mul(t4, s_all[:, :, 1, :], kpr)
            po = pre_all[:, b]
            qo = pim_all[:, b]
            # K.im negated for B chunks on host -> uniform signs; B-chunk
            # pim is stored as -Im(P) (handled in UD im-swap + bhnr rows)
            nc.vector.tensor_sub(po, t1, t2)
            nc.vector.tensor_add(qo, t3, t4)
            return xt

        rep_ctx = tc.For_i(0, reps, 1) if reps > 1 else None
        if rep_ctx is not None:
            rep_ctx.__enter__()
        for ch in range(n_ch):
            pre_all = p_p.tile([128, n_b, 4, WFP], BF16, tag="pre")
            pim_all = p_p.tile([128, n_b, 4, WFP], BF16, tag="pim")
            t256b = ny_p.tile([128, 2, n_b], BF16, tag="t256")
            xts = []
            for b in range(n_b):
                xts.append(fwd(xs[ch * n_b + b], ch, b, pre_all, pim_all, t256b))

            # ---- NyqH (hf=256): batched transposed spectrum + R rows ----
            s256p = psd_p.tile([128, 2, 2, n_b], F32, tag="psd")
            for c in range(2):
                cs = slice(c * 128, (c + 1) * 128)
                for kw in range(2):
                    MM(s256p[:, c, 0, :], awr[:, kw, cs], t256b[:, kw, :],
                       start=(kw == 0), stop=(kw == 1))
                for kw in range(2):
                    MM(s256p[:, c, 1, :], awi[:, kw, cs], t256b[:, kw, :],
                       start=(kw == 0), stop=(kw == 1))
            s256 = ny_p.tile([128, 2, 2, n_b], BF16, tag="s256")
            nc.scalar.copy(out=s256, in_=s256p)
            p256 = ny_p.tile([128, 2, 2, n_b], BF16, tag="p256")
            for c in range(2):
                m2 = ny_p.tile([128, 2, n_b], BF16, tag="m2")
                krs = k2t[:, ch, 2 * c:2 * c + 1]       # K256T re, chunk c
                kis = k2t[:, ch, 2 * c + 1:2 * c + 2]   # K256T im, chunk c
                nc.vector.tensor_scalar_mul(m2[:, 0, :], s256[:, c, 1, :], kis)
                nc.vector.scalar_tensor_tensor(
                    p256[:, c, 0, :], s256[:, c, 0, :], krs, m2[:, 0, :],
                    MULT, SUB)
                nc.vector.tensor_scalar_mul(m2[:, 1, :], s256[:, c, 1, :], krs)
                nc.vector.scalar_tensor_tensor(
                    p256[:, c, 1, :], s256[:, c, 0, :], kis, m2[:, 1, :],
                    MULT, ADD)
            rp = psd_p.tile([n_b, W], F32, tag="psd")
            for c in range(2):
                MM(rp, p256[:, c, 0, :], gc[:, c, :],
                   start=(c == 0), stop=False)
                MM(rp, p256[:, c, 1, :], gsn[:, c, :],
                   start=False, stop=(c == 1))
            rsb = ny_p.tile([n_b, W], BF16, tag="rsb")
            nc.scalar.copy(out=rsb, in_=rp)
            r0s = []
            for b in range(n_b):
                r0 = r0_p.tile([1, W], BF16, tag="r0")
                nc.sync.dma_start(out=r0, in_=rsb[b:b + 1, :])
                r0s.append(r0)

            # ---- vnyq (wf=256): batched for the wave ----
            pvnT = psd_p.tile([128, 2, n_b], F32, tag="psd")
            for mh in range(2):
                for khf in range(4):
                    MM(pvnT[:, mh, :], bhri[:, khf, slice(mh * 128, (mh + 1) * 128)],
                       pre_all[:, :, khf, 256], start=(khf == 0), stop=False)
                    MM(pvnT[:, mh, :], bhnr[:, khf, slice(mh * 128, (mh + 1) * 128)],
                       pim_all[:, :, khf, 256], start=False, stop=(khf == 3))
            vnyqT = v_p.tile([128, 2, n_b], BF16, tag="vnyqT")
            nc.scalar.copy(out=vnyqT, in_=pvnT)

            # ---- U/D pre-adds + inverse transforms + output per plane ----
            for b in range(n_b):
                pl = ch * n_b + b
                ure = ud_p.tile([128, 2, WFP], BF16, tag="ure")
                dre = ud_p.tile([128, 2, WFP], BF16, tag="dre")
                uim = ud_p.tile([128, 2, WFP], BF16, tag="uim")
                dim = ud_p.tile([128, 2, WFP], BF16, tag="dim")
                pa_re = pre_all[:, b, 0:2, :]
                pb_re = pre_all[:, b, 2:4, :]
                pa_im = pim_all[:, b, 0:2, :]
                pb_im = pim_all[:, b, 2:4, :]
                nc.vector.tensor_add(ure, pa_re, pb_re)
                nc.vector.tensor_sub(dre, pa_re, pb_re)
                # B-chunk pim holds -Im(P): swap add/sub for the im parts
                nc.gpsimd.tensor_sub(uim, pa_im, pb_im)
                nc.gpsimd.tensor_add(dim, pa_im, pb_im)
                if debug and ch == 0 and b == 0:
                    dbt = y_p.tile([128, 4, WFP], F32, tag="dbt")
                    nc.vector.tensor_copy(dbt[:, 0:2], pre_all[:, b, 0:2, :])
                    nc.vector.tensor_copy(dbt[:, 2:4], pre_all[:, b, 2:4, :])
                    nc.sync.dma_start(out=dpre, in_=dbt)
                    dbt2 = y_p.tile([128, 4, WFP], F32, tag="dbt2")
                    nc.vector.tensor_copy(dbt2[:, 0:2], pim_all[:, b, 0:2, :])
                    nc.vector.tensor_copy(dbt2[:, 2:4], pim_all[:, b, 2:4, :])
                    nc.sync.dma_start(out=dpim, in_=dbt2)
                    dbu = y_p.tile([128, 4, 2, WFP], F32, tag="dbu")
                    nc.vector.tensor_copy(dbu[:, 0], ure)
                    nc.vector.tensor_copy(dbu[:, 1], dre)
                    nc.vector.tensor_copy(dbu[:, 2], uim)
                    nc.vector.tensor_copy(dbu[:, 3], dim)
                    nc.sync.dma_start(out=dud, in_=dbu)

                v_both = v_p.tile([128, 2, 2, W], BF16, tag="vb")
                for mwf in range(2):
                    vp = ps3_p.tile([128, 2, W], F32, tag="ps3")
                    ms = slice(mwf * 128, (mwf + 1) * 128)
                    # each psum region's accumulation run must be contiguous
                    for pair in range(2):
                        MM(vp[:, 0, :], ure[:, pair, ms], crp[:, pair, :],
                           start=(pair == 0), stop=False)
                        MM(vp[:, 0, :], dim[:, pair, ms], cin[:, pair, :],
                           start=False, stop=(pair == 1))
                    for pair in range(2):
                        MM(vp[:, 1, :], dre[:, pair, ms], cip[:, pair, :],
                           start=(pair == 0), stop=False)
                        MM(vp[:, 1, :], uim[:, pair, ms], crp[:, pair, :],
                           start=False, stop=(pair == 1))
                    nc.scalar.copy(out=v_both[:, mwf], in_=vp)
                if debug and ch == 0 and b == 0:
                    dbv = y_p.tile([128, 2, 2, W], F32, tag="dbv")
                    nc.vector.tensor_copy(dbv, v_both)
                    nc.sync.dma_start(out=dvb, in_=dbv)

                ysb = y_p.tile([128, 2, W], BF16, tag="ysb")
                xt = xts[b]
                for mh in range(2):
                    py = psd_p.tile([128, W], F32, tag="psd")
                    mhs = slice(mh * 128, (mh + 1) * 128)
                    MM(py, v_both[:, 0, 0, mhs], gc[:, 0, :],
                       start=True, stop=False)
                    MM(py, v_both[:, 0, 1, mhs], gsn[:, 0, :],
                       start=False, stop=False)
                    MM(py, v_both[:, 1, 0, mhs], gc[:, 1, :],
                       start=False, stop=False)
                    MM(py, v_both[:, 1, 1, mhs], gsn[:, 1, :],
                       start=False, stop=False)
                    MM(py, sgn1, r0s[b], start=False, stop=True)
                    tny = tm_p.tile([128, W], F32, tag="tm")
                    nc.vector.scalar_tensor_tensor(
                        tny, pm1r, vnyqT[:, mh, b:b + 1], xt[:, mh, :],
                        MULT, ADD)
                    nc.vector.tensor_add(ysb[:, mh, :], tny, py)
                nc.sync.dma_start(out=ys[pl].rearrange("(k p) w -> p k w", p=128),
                                  in_=ysb)
        if rep_ctx is not None:
            rep_ctx.__exit__(None, None, None)
    _dedupe_ldweights(nc)
    _legalize_waits(nc)
    return nc


def filter_spectra(filt_slice: np.ndarray):
    """Host rfft2 -> chunk layout [128, n_ch, 4, WFP] (re, im) with B0 row-0
    zeroed, plus K[256,:] transposed chunks [128, n_ch, 4] = (c0re, c0im,
    c1re, c1im)."""
    n_ch = filt_slice.shape[0]
    kf = np.fft.rfft2(filt_slice.astype(np.float64), s=(N, N))  # [n_ch,512,257]
    k256 = kf[:, 256, :]                                       # [n_ch, 257]
    kc = kf[:, _hfidx(), :]
    kc[:, 256] = 0.0          # duplicate hf=0 row (B0 chunk row 0)
    kk = kc.reshape(n_ch, 4, 128, WF).transpose(2, 0, 1, 3)    # [128,n_ch,4,257]
    out = np.zeros((2, 128, n_ch, 4, WFP), np.float32)
    out[0, :, :, :, :WF] = kk.real
    out[1, :, :, :, :WF] = kk.imag
    out[1, :, :, 2:4, :] *= -1.0    # fold B-chunk conj sign into K.im
    k2 = np.zeros((128, n_ch, 4), np.float32)
    k2[:, :, 0] = k256.real[:, 0:128].T
    k2[:, :, 1] = k256.imag[:, 0:128].T
    k2[:, :, 2] = k256.real[:, 128:256].T
    k2[:, :, 3] = k256.imag[:, 128:256].T
    return (out[0].astype(NPBF16), out[1].astype(NPBF16), k2)


def kernel(x: np.ndarray, filt: np.ndarray) -> np.ndarray:
    x = np.ascontiguousarray(x, dtype=np.float32)
    xb = x.astype(NPBF16)
    filt = np.ascontiguousarray(filt, dtype=np.float32)
    cblob = _consts()[0]
    nc = build_nc()
    in_maps = []
    for i in range(NCORES):
        sl = slice(i * CPC, (i + 1) * CPC)
        xsh = np.ascontiguousarray(
            xb[:, sl].transpose(1, 0, 2, 3).reshape(PLANES, H, W))
        kr, ki, k2 = filter_spectra(filt[sl])
        in_maps.append({"xs": xsh, "kr": kr, "ki": ki, "k2": k2,
                        "cblob": cblob})
    res = run_bass_kernel_spmd(nc, in_maps, core_ids=list(range(NCORES)))
    out = np.empty_like(x)
    for i in range(NCORES):
        sl = slice(i * CPC, (i + 1) * CPC)
        out[:, sl] = res.results[i]["ys"].astype(np.float32).reshape(CPC, B, H, W).transpose(1, 0, 2, 3)
    return out
